# revision 4
# baseline (speedup 1.0000x reference)
"""RegionLoss (YOLOv2) filter kernel v2 — fp16 datapath, (x,a) layout.

Shapes: output (16,425,64,64) f32, target (16,50,5) f32, anchors (5,2) f32.
A=5, C=80, H=W=64, N=50, STRIDE=16. 8 cores, 2 batches each.

Device computes a conservative candidate filter vres[p=(b,y), f=(x,a)]:
  vres = max_n [ relu(dx_n)*dy_n + c5_n ] - 0.375*pa  (>0 => candidate)
with per-gt row-packed slots (S slots, x-windows). Host does the exact
fp32 tail (iou/argmax/loss) on the ~2k candidates.

v2 vs v1: host pre-packs the 20 coord channels to fp16 in device layout
(halves DMA bytes, makes everything contiguous); free dim is (x,a) so slot
windows are contiguous (fp16 2x/4x DVE modes engage); per-slot work is
spread ACT/DVE/GPSIMD; consts built on-device (iota/memset, no 491KB DMA);
DMAs issued from 3 queues; output DMA split 4 ways.
"""

import os
import numpy as np

import concourse.bass as bass
import concourse.mybir as mybir
from concourse import tile
from concourse.bass_utils import run_bass_kernel_spmd
from concourse.vector_clock import ScopedClock
import bass_rust

F32 = mybir.dt.float32
F16 = mybir.dt.float16
OP = mybir.AluOpType
AF = mybir.ActivationFunctionType

A, C, H, W, N = 5, 80, 64, 64, 50
B = 16
NCORES = 8
BPC = B // NCORES
STRIDE = 16.0
THRESH = 0.6
T375 = THRESH / (1.0 + THRESH)
NULL_C5 = -1.0e9
XSHIFT = 32.0
VM_INIT = -60000.0


# ---------------------------------------------------------------------------
# Tile tail-drain patch + multi-wait splitting (same as v1): cheap teardown.
# ---------------------------------------------------------------------------
def _patched_drain_and_barrier(self, tick_clock, wait_clock):
    nc = self.nc
    drain_inst = nc.sync.drain()
    wait_clock.add_sem_waits(drain_inst.ins, ScopedClock({None: tick_clock.global_clock}))
    si = drain_inst.ins.sync_info
    if si is not None and len(si.on_wait) > 1:
        waits = list(si.on_wait)
        drain_inst.ins.sync_info = bass_rust.SyncInfo(
            on_wait=[waits[0]], on_update=list(si.on_update)
        )
        for w in waits[1:]:
            nop = nc.sync.nop(nofuse=True)
            nop.ins.sync_info = bass_rust.SyncInfo(on_wait=[w], on_update=[])

    assert self.sems is not None
    popped = nc._tile_sem_poison_stack.pop()
    assert popped is self._sem_poison

    from concourse.bass import compact_to_ranges

    sems = list(self.sems.allocated().values())
    if sems:
        hs = nc._state.alloc_semaphore(name="td_hs")
        nc.sync.sem_inc(hs, 1)
        nc.gpsimd.wait_ge(hs, 1)
        sem_nums = [s.num if hasattr(s, "num") else s for s in sems] + [
            hs.num if hasattr(hs, "num") else hs
        ]
        for sem_range in compact_to_ranges(sorted(sem_nums)):
            nc.gpsimd.dma_reset(sem_range)
            nc.gpsimd.sem_clear(sem_range)
        nc._state.prepend_free_semaphores(sem_nums)
        for poison_set in nc._tile_sem_poison_stack:
            poison_set.update(sem_nums)


if not os.environ.get("K2_NO_PATCH") and getattr(tile.TileContext, "_drain_patch", None) is None:
    tile.TileContext._drain_and_barrier = _patched_drain_and_barrier
    tile.TileContext._drain_patch = True


def _make_wait_nop(nc, engine_type, w):
    nop = nc.engines[engine_type].nop(nofuse=True)
    inst = nop.ins
    cur = nc.cur_bb.bb
    lst = list(cur.instructions)
    assert lst and lst[-1].name == inst.name, "nop not at tail of cur_bb"
    cur.instructions = lst[:-1]
    inst.sync_info = bass_rust.SyncInfo(on_wait=[w], on_update=[])
    return inst


def _split_multiwait(nc):
    for f in nc.m.functions:
        for bb in f.blocks:
            insts = list(bb.instructions)
            out = []
            changed = False
            for ins in insts:
                si = ins.sync_info
                cap = 2 if isinstance(ins, mybir.InstEventSemaphore) else 1
                if si is not None and len(si.on_wait) > cap:
                    changed = True
                    waits = list(si.on_wait)
                    for w in waits[:-cap]:
                        out.append(_make_wait_nop(nc, ins.engine, w))
                    ins.sync_info = bass_rust.SyncInfo(
                        on_wait=waits[-cap:], on_update=list(si.on_update)
                    )
                out.append(ins)
            if changed:
                bb.instructions = out


# ---------------------------------------------------------------------------
# Device program
# ---------------------------------------------------------------------------
_NC_CACHE = {}
ANCHORS = np.array([[18.3, 21.6], [60.0, 66.0], [106.8, 175.5],
                    [252.2, 112.9], [312.7, 293.4]], np.float32)


def _build_nc(S, geo):
    nc = bass.Bass()
    slab = nc.dram_tensor("slab", [128, 4, 320], F16, kind="ExternalInput")
    gtt = nc.dram_tensor("gtt", [128, 5 * S], F32, kind="ExternalInput")
    vout = nc.dram_tensor("vout", [128, 320], F16, kind="ExternalOutput")

    with tile.TileContext(nc) as tc:
        with (
            tc.tile_pool(name="cpool", bufs=1) as cpool,
            tc.tile_pool(name="wpool", bufs=1) as wpool,
            tc.tile_pool(name="lpool", bufs=6) as lpool,
        ):
            # ---- input DMAs: planes TX,TY,TW,TH split by partition half ----
            # HWDGE queues are SP + Activation; ACT issues before its compute.
            T16 = wpool.tile([128, 4 * 320], F16)
            for c in range(4):
                lo = T16[0:64, 320 * c:320 * (c + 1)]
                hi = T16[64:128, 320 * c:320 * (c + 1)]
                nc.sync.dma_start(lo, slab[0:64, c, :])
                nc.scalar.dma_start(hi, slab[64:128, c, :])

            # ---- on-device consts (gpsimd) + gt table DMA (swdge) ----
            XOFF2 = cpool.tile([128, 320], F16)
            XOFFH = cpool.tile([128, 320], F16)
            AW2W = cpool.tile([128, 320], F16)
            AH2W = cpool.tile([128, 320], F16)
            # 2x-63 (ints, exact in fp16), halved -> x-31.5
            nc.gpsimd.iota(XOFF2[:], [[2, 64], [0, 5]], base=-63,
                           channel_multiplier=0,
                           allow_small_or_imprecise_dtypes=True)
            nc.gpsimd.tensor_scalar_mul(XOFFH[:], XOFF2[:], 0.5)
            aw = (ANCHORS[:, 0] / 32.0).astype(np.float32)
            ah = (ANCHORS[:, 1] / 32.0).astype(np.float32)
            AW2v = AW2W[:].rearrange("p (x a) -> p a x", a=A)
            AH2v = AH2W[:].rearrange("p (x a) -> p a x", a=A)
            for a in range(A):
                nc.gpsimd.memset(AW2v[:, a, :], float(aw[a]))
                nc.gpsimd.memset(AH2v[:, a, :], float(ah[a]))
            GTT = cpool.tile([128, 5 * S], F32)
            nc.gpsimd.dma_start(GTT[:], gtt[:])
            VM = wpool.tile([128, 320], F16)
            nc.vector.memset(VM[:], VM_INIT)

            TX = T16[:, 0:320]
            TY = T16[:, 320:640]
            TW = T16[:, 640:960]
            TH = T16[:, 960:1280]

            # ---- decode: sigmoid via tanh (one ACT table set for all fns) ----
            # sig(t) = 0.5*tanh(0.5 t) + 0.5; the 0.5-shift is folded into
            # XOFFH (x side) and the host gt scalars (y side).
            TXh = wpool.tile([128, 320], F16)
            TYh = wpool.tile([128, 320], F16)
            E0 = wpool.tile([128, 320], F16)
            E1 = wpool.tile([128, 320], F16)
            nc.scalar.activation(TXh[:], TX, AF.Tanh, scale=0.5)
            nc.scalar.activation(TYh[:], TY, AF.Tanh, scale=0.5)
            nc.scalar.activation(E0[:], TW, AF.Exp)
            nc.scalar.activation(E1[:], TH, AF.Exp)

            EW = wpool.tile([128, 320], F16)
            EH = wpool.tile([128, 320], F16)
            SXO = wpool.tile([128, 320], F16)
            NX1 = wpool.tile([128, 320], F16)
            PX2 = wpool.tile([128, 320], F16)
            NY1 = wpool.tile([128, 320], F16)
            PY2 = wpool.tile([128, 320], F16)
            NPA = wpool.tile([128, 320], F16)
            # SXO = 0.5*TXh + (x-31.5) = sig(tx) + x - 32
            nc.vector.scalar_tensor_tensor(SXO[:], TXh[:], 0.5, XOFFH[:], OP.mult, OP.add)
            nc.vector.tensor_mul(EW[:], E0[:], AW2W[:])
            nc.vector.tensor_sub(NX1[:], EW[:], SXO[:])
            nc.vector.tensor_add(PX2[:], SXO[:], EW[:])
            nc.vector.tensor_mul(EH[:], E1[:], AH2W[:])
            # y side carries a -0.5 shift (folded into host gy scalars):
            # PY2' = EH + 0.5*TYh = py2 - 0.5 ; NY1' = EH - 0.5*TYh = ny1 + 0.5
            nc.vector.scalar_tensor_tensor(PY2[:], TYh[:], 0.5, EH[:], OP.mult, OP.add)
            nc.vector.scalar_tensor_tensor(NY1[:], TYh[:], -0.5, EH[:], OP.mult, OP.add)
            # npa = -1.5 * EW * EH  ( = -0.375 * pa, pa = 4*EW*EH )
            nc.vector.scalar_tensor_tensor(NPA[:], EW[:], -1.5, EH[:], OP.mult, OP.mult)

            def gcol(k, s):
                return GTT[:, k * S + s: k * S + s + 1]

            def win(t, s):
                xlo, wdt = geo[s]
                return t[:, 5 * xlo: 5 * (xlo + wdt)]

            # ---- slot loop (software pipelined) ----
            st1, st2, st3 = {}, {}, {}

            def emit_stage1(s):
                fd = 5 * geo[s][1]
                r1x = lpool.tile([128, fd], F16, name=f"r1x_{s}", tag="r1x")
                r1y = lpool.tile([128, fd], F16, name=f"r1y_{s}", tag="r1y")
                u = lpool.tile([128, fd], F16, name=f"u_{s}", tag="u")
                v = lpool.tile([128, fd], F16, name=f"v_{s}", tag="v")
                nc.scalar.activation(r1x[:], win(PX2, s), AF.Relu, bias=gcol(0, s), scale=-1.0)
                nc.scalar.activation(r1y[:], win(PY2, s), AF.Relu, bias=gcol(2, s), scale=-1.0)
                nc.vector.tensor_scalar(u[:], win(NX1, s), gcol(1, s), gcol(0, s), OP.min, OP.add)
                nc.vector.tensor_scalar(v[:], win(NY1, s), gcol(3, s), gcol(2, s), OP.min, OP.add)
                st1[s] = (r1x, r1y, u, v)

            def emit_stage2(s):
                r1x, r1y, u, v = st1.pop(s)
                fd = 5 * geo[s][1]
                dx = lpool.tile([128, fd], F16, name=f"dx_{s}", tag="dx")
                dy = lpool.tile([128, fd], F16, name=f"dy_{s}", tag="dy")
                nc.gpsimd.tensor_sub(dx[:], u[:], r1x[:])
                nc.gpsimd.tensor_sub(dy[:], v[:], r1y[:])
                st2[s] = (dx, dy)

            def emit_stage3(s):
                dx, dy = st2.pop(s)
                fd = 5 * geo[s][1]
                iv = lpool.tile([128, fd], F16, name=f"iv_{s}", tag="iv")
                nc.vector.scalar_tensor_tensor(iv[:], dx[:], 0.0, dy[:], OP.max, OP.mult)
                st3[s] = iv

            def emit_stage4(s):
                iv = st3.pop(s)
                nc.vector.scalar_tensor_tensor(
                    win(VM, s), iv[:], gcol(4, s), win(VM, s), OP.add, OP.max
                )

            for s in range(S + 3):
                if s < S:
                    emit_stage1(s)
                if s >= 1 and s - 1 < S:
                    emit_stage2(s - 1)
                if s >= 2 and s - 2 < S:
                    emit_stage3(s - 2)
                if s >= 3:
                    emit_stage4(s - 3)

            VR = wpool.tile([128, 320], F16)
            nc.vector.tensor_add(VR[:], VM[:], NPA[:])
            nc.sync.dma_start(vout[0:43, :], VR[0:43, :])
            nc.scalar.dma_start(vout[43:86, :], VR[43:86, :])
            nc.gpsimd.dma_start(vout[86:128, :], VR[86:128, :])

    _split_multiwait(nc)
    return nc


def _get_nc(S, geo):
    key = (S, tuple(geo))
    if key not in _NC_CACHE:
        _NC_CACHE[key] = _build_nc(S, geo)
    return _NC_CACHE[key]


# ---------------------------------------------------------------------------
# Host: geometry, packing, tables
# ---------------------------------------------------------------------------
def _gt_geom(target):
    tgt = target.astype(np.float32)
    inv16 = np.float32(1.0 / 16.0)
    cx = tgt[:, :, 1] * inv16
    cy = tgt[:, :, 2] * inv16
    w = tgt[:, :, 3] * inv16
    h = tgt[:, :, 4] * inv16
    gx1 = cx - w * np.float32(0.5)
    gx2 = cx + w * np.float32(0.5)
    gy1 = cy - h * np.float32(0.5)
    gy2 = cy + h * np.float32(0.5)
    ga = w * h
    return gx1, gx2, gy1, gy2, w, h, ga


def _delta(w, h, ga):
    # fp16 pipeline rounding (~0.05*(gw+gh)) + ACT table slop + pa-term error.
    return (np.float32(0.05) * (w + h) + np.float32(0.016) * ga
            + np.float32(0.02)).astype(np.float32)


def _pack(items, wgrow_cap):
    items = sorted(items, key=lambda t: (t[5], t[6]))
    slots = []
    for it in items:
        core, b, n, y0, y1, x0, x1 = it
        mask = ((1 << (y1 - y0 + 1)) - 1) << (64 * b + y0)
        best, best_cost = -1, None
        for si, sl in enumerate(slots):
            if sl[2].get(core, 0) & mask:
                continue
            grow = max(sl[1], x1) - min(sl[0], x0) - (sl[1] - sl[0])
            if best_cost is None or grow < best_cost:
                best, best_cost = si, grow
        if best < 0 or best_cost > wgrow_cap:
            slots.append([x0, x1, {core: mask}, [it]])
        else:
            sl = slots[best]
            sl[0] = min(sl[0], x0)
            sl[1] = max(sl[1], x1)
            sl[2][core] = sl[2].get(core, 0) | mask
            sl[3].append(it)
    return [(sl[0], sl[1], sl[3]) for sl in slots]


def _host_tables(target, anchors):
    gx1, gx2, gy1, gy2, w, h, ga = _gt_geom(target)
    delta = _delta(w, h, ga)
    d_rel = delta / ga
    E = np.maximum(
        0.0,
        np.float32(0.125) * np.minimum(np.float32(2.667),
                                       np.float32(1.667) + np.float32(2.667) * d_rel)
        - np.float32(T375) + d_rel,
    ).astype(np.float32)
    PAD = 0.10

    def cell_range(lo, hi, ext):
        c0 = np.clip(np.floor(lo - ext - PAD + 1.0) - 1.0, 0, 63).astype(np.int64)
        c1 = np.clip(np.ceil(hi + ext + PAD) - 1.0, 0, 63).astype(np.int64)
        return c0, np.maximum(c1, c0)

    y0c, y1c = cell_range(gy1, gy2, E * h)
    x0c, x1c = cell_range(gx1, gx2, E * w)

    items = []
    for i in range(NCORES):
        for b in range(BPC):
            g = 2 * i + b
            for n in range(N):
                items.append((i, b, n, int(y0c[g, n]), int(y1c[g, n]),
                              int(x0c[g, n]), int(x1c[g, n])))
    best = None
    for cap in (24, 32, 48):
        slots = _pack(items, cap)
        cost = sum(min(64, ((sl[1] - sl[0] + 1 + 15) // 16) * 16) + 32 for sl in slots)
        if best is None or cost < best[0]:
            best = (cost, slots)
    slots = best[1]
    S = len(slots)
    geo = []
    for (xlo, xhi, _) in slots:
        xlo2 = (xlo // 8) * 8
        wdt = min(64 - xlo2, ((xhi - xlo2 + 1 + 15) // 16) * 16)
        geo.append((int(xlo2), int(wdt)))

    c5 = (-np.float32(T375) * ga + delta).astype(np.float32)
    yrow = np.arange(64, dtype=np.float32)
    gtts = [np.zeros((128, 5 * S), np.float32) for _ in range(NCORES)]
    for i in range(NCORES):
        gtts[i][:, 4 * S:5 * S] = NULL_C5
    for s, (_, _, members) in enumerate(slots):
        for (i, b, n, r0, r1, _, _) in members:
            g = 2 * i + b
            rows = slice(64 * b + r0, 64 * b + r1 + 1)
            yv = yrow[r0:r1 + 1]
            gtts[i][rows, 0 * S + s] = gx2[g, n] - XSHIFT
            gtts[i][rows, 1 * S + s] = -(gx1[g, n] - XSHIFT)
            # y side: device PY2/NY1 carry a -0.5 shift (tanh-sigmoid fold)
            gtts[i][rows, 2 * S + s] = gy2[g, n] - yv - 0.5
            gtts[i][rows, 3 * S + s] = yv - gy1[g, n] + 0.5
            gtts[i][rows, 4 * S + s] = c5[g, n]
    return gtts, S, geo


def _prep_slabs(output):
    """(16,425,64,64) f32 -> per-core [128, 4, 320] fp16, planes TX,TY,TW,TH,
    free dim (x,a)."""
    o = output.reshape(B, A, 85, H, W)[:, :, :4]        # (B, a, c, y, x)
    arr = np.ascontiguousarray(o.transpose(0, 3, 2, 4, 1))  # (B, y, c, x, a)
    arr = arr.reshape(B, 64, 4, 320).astype(np.float16)
    return [np.ascontiguousarray(
        np.concatenate([arr[2 * i], arr[2 * i + 1]], axis=0)) for i in range(NCORES)]


# ---------------------------------------------------------------------------
# Exact host tail (same as v1)
# ---------------------------------------------------------------------------
def _sigmoid32(x):
    return np.float32(1.0) / (np.float32(1.0) + np.exp(-x, dtype=np.float32))


def _exact_candidates(output, target, anchors, cand_idx):
    bg, aa, yy, xx = cand_idx
    if bg.shape[0] == 0:
        z = np.zeros(0)
        return z.astype(bool), z.astype(np.int64)

    out = output
    tx = out[bg, 85 * aa + 0, yy, xx]
    ty = out[bg, 85 * aa + 1, yy, xx]
    tw = out[bg, 85 * aa + 2, yy, xx]
    th = out[bg, 85 * aa + 3, yy, xx]
    an = anchors.astype(np.float32)
    px = (_sigmoid32(tx) + xx.astype(np.float32)) * np.float32(STRIDE)
    py = (_sigmoid32(ty) + yy.astype(np.float32)) * np.float32(STRIDE)
    pw = np.exp(tw, dtype=np.float32) * an[aa, 0]
    ph = np.exp(th, dtype=np.float32) * an[aa, 1]

    g = target[:, :, 1:].astype(np.float32)
    gx1 = g[:, :, 0] - g[:, :, 2] * np.float32(0.5)
    gx2 = g[:, :, 0] + g[:, :, 2] * np.float32(0.5)
    gy1 = g[:, :, 1] - g[:, :, 3] * np.float32(0.5)
    gy2 = g[:, :, 1] + g[:, :, 3] * np.float32(0.5)
    g_area = (gx2 - gx1) * (gy2 - gy1)

    px1 = px - pw * np.float32(0.5)
    px2 = px + pw * np.float32(0.5)
    py1 = py - ph * np.float32(0.5)
    py2 = py + ph * np.float32(0.5)
    p_area = (px2 - px1) * (py2 - py1)

    ix1 = np.maximum(gx1[bg], px1[:, None])
    iy1 = np.maximum(gy1[bg], py1[:, None])
    ix2 = np.minimum(gx2[bg], px2[:, None])
    iy2 = np.minimum(gy2[bg], py2[:, None])
    inter = np.clip(ix2 - ix1, 0, None) * np.clip(iy2 - iy1, 0, None)
    union = g_area[bg] + p_area[:, None] - inter + np.float32(1e-6)
    iou = inter / union
    best = iou.max(axis=1)
    bidx = iou.argmax(axis=1)
    return best > np.float32(THRESH), bidx


def _build_run_args(output, target, anchors):
    """(nc, in_maps) for the device run — shared with the test harness."""
    gtts, S, geo = _host_tables(target, anchors)
    slabs = _prep_slabs(output)
    nc = _get_nc(S, geo)
    in_maps = [{"slab": slabs[i], "gtt": gtts[i]} for i in range(NCORES)]
    return nc, in_maps


LAST_VMAX = None  # test-harness introspection


def kernel(output, target, anchors):
    global LAST_VMAX
    output = np.ascontiguousarray(output, np.float32)
    target = np.ascontiguousarray(target, np.float32)
    anchors = np.ascontiguousarray(anchors, np.float32)

    nc, in_maps = _build_run_args(output, target, anchors)
    res = run_bass_kernel_spmd(nc, in_maps, list(range(NCORES)))

    vmax = np.zeros((B, A, H, W), np.float32)
    for i in range(NCORES):
        vo = res.results[i]["vout"].astype(np.float32)
        for b in range(BPC):
            g = 2 * i + b
            vmax[g] = (
                vo[64 * b:64 * b + 64, :].reshape(64, 64, 5).transpose(2, 0, 1)
            )
    LAST_VMAX = vmax

    cand = vmax > 0.0
    bg, aa, yy, xx = np.nonzero(cand)
    mask_c, bidx_c = _exact_candidates(output, target, anchors, (bg, aa, yy, xx))

    m = mask_c
    bgm, aam, yym, xxm = bg[m], aa[m], yy[m], xx[m]
    idxm = bidx_c[m]

    coord_loss = 0.0
    if bgm.size:
        d = 0.0
        for c in range(4):
            pc = output[bgm, 85 * aam + c, yym, xxm].astype(np.float64)
            tc = target[bgm, idxm, 1 + c].astype(np.float64)
            d += np.sum((pc - tc) ** 2)
        coord_loss = d

    conf_all = output[:, 4::85, :, :].astype(np.float64)
    conf_loss = np.sum(conf_all * conf_all)
    if bgm.size:
        cm = output[bgm, 85 * aam + 4, yym, xxm].astype(np.float64)
        conf_loss += np.sum(25.0 * (cm - 1.0) ** 2 - cm * cm)

    cls_loss = 0.0
    if bgm.size:
        ch = (85 * aam[:, None] + 5 + np.arange(C)[None, :])
        logits = output[bgm[:, None], ch, yym[:, None], xxm[:, None]].astype(np.float64)
        lse = np.log(np.sum(np.exp(logits), axis=1))
        tcls = target[bgm, idxm, 0].astype(np.int64)
        logit_sel = logits[np.arange(bgm.size), tcls]
        cls_loss = np.sum(lse - logit_sel)

    total = coord_loss + conf_loss + cls_loss
    return np.float32(total)


# revision 8
# speedup vs baseline: 1.0751x; 1.0751x over previous
"""RegionLoss (YOLOv2) filter kernel v2 — fp16 datapath, (x,a) layout.

Shapes: output (16,425,64,64) f32, target (16,50,5) f32, anchors (5,2) f32.
A=5, C=80, H=W=64, N=50, STRIDE=16. 8 cores, 2 batches each.

Device computes a conservative candidate filter vres[p=(b,y), f=(x,a)]:
  vres = max_n [ relu(dx_n)*dy_n + c5_n ] - 0.375*pa  (>0 => candidate)
with per-gt row-packed slots (S slots, x-windows). Host does the exact
fp32 tail (iou/argmax/loss) on the ~2k candidates.

v2 vs v1: host pre-packs the 20 coord channels to fp16 in device layout
(halves DMA bytes, makes everything contiguous); free dim is (x,a) so slot
windows are contiguous (fp16 2x/4x DVE modes engage); per-slot work is
spread ACT/DVE/GPSIMD; consts built on-device (iota/memset, no 491KB DMA);
DMAs issued from 3 queues; output DMA split 4 ways.
"""

import os
import numpy as np

import concourse.bass as bass
import concourse.mybir as mybir
from concourse import tile
from concourse.bass_utils import run_bass_kernel_spmd
from concourse.vector_clock import ScopedClock
import bass_rust

F32 = mybir.dt.float32
F16 = mybir.dt.float16
OP = mybir.AluOpType
AF = mybir.ActivationFunctionType

A, C, H, W, N = 5, 80, 64, 64, 50
B = 16
NCORES = 8
BPC = B // NCORES
STRIDE = 16.0
THRESH = 0.6
T375 = THRESH / (1.0 + THRESH)
NULL_C5 = -1.0e9
XSHIFT = 32.0
VM_INIT = -60000.0


# ---------------------------------------------------------------------------
# Tile tail-drain patch + multi-wait splitting (same as v1): cheap teardown.
# ---------------------------------------------------------------------------
def _patched_drain_and_barrier(self, tick_clock, wait_clock):
    nc = self.nc
    drain_inst = nc.sync.drain()
    wait_clock.add_sem_waits(drain_inst.ins, ScopedClock({None: tick_clock.global_clock}))
    si = drain_inst.ins.sync_info
    if si is not None and len(si.on_wait) > 1:
        waits = list(si.on_wait)
        drain_inst.ins.sync_info = bass_rust.SyncInfo(
            on_wait=[waits[0]], on_update=list(si.on_update)
        )
        for w in waits[1:]:
            nop = nc.sync.nop(nofuse=True)
            nop.ins.sync_info = bass_rust.SyncInfo(on_wait=[w], on_update=[])

    assert self.sems is not None
    popped = nc._tile_sem_poison_stack.pop()
    assert popped is self._sem_poison

    from concourse.bass import compact_to_ranges

    sems = list(self.sems.allocated().values())
    if sems:
        hs = nc._state.alloc_semaphore(name="td_hs")
        nc.sync.sem_inc(hs, 1)
        nc.gpsimd.wait_ge(hs, 1)
        sem_nums = [s.num if hasattr(s, "num") else s for s in sems] + [
            hs.num if hasattr(hs, "num") else hs
        ]
        for sem_range in compact_to_ranges(sorted(sem_nums)):
            nc.gpsimd.dma_reset(sem_range)
            nc.gpsimd.sem_clear(sem_range)
        nc._state.prepend_free_semaphores(sem_nums)
        for poison_set in nc._tile_sem_poison_stack:
            poison_set.update(sem_nums)


if not os.environ.get("K2_NO_PATCH") and getattr(tile.TileContext, "_drain_patch", None) is None:
    tile.TileContext._drain_and_barrier = _patched_drain_and_barrier
    tile.TileContext._drain_patch = True


def _make_wait_nop(nc, engine_type, w):
    nop = nc.engines[engine_type].nop(nofuse=True)
    inst = nop.ins
    cur = nc.cur_bb.bb
    lst = list(cur.instructions)
    assert lst and lst[-1].name == inst.name, "nop not at tail of cur_bb"
    cur.instructions = lst[:-1]
    inst.sync_info = bass_rust.SyncInfo(on_wait=[w], on_update=[])
    return inst


def _split_multiwait(nc):
    for f in nc.m.functions:
        for bb in f.blocks:
            insts = list(bb.instructions)
            out = []
            changed = False
            for ins in insts:
                si = ins.sync_info
                cap = 2 if isinstance(ins, mybir.InstEventSemaphore) else 1
                if si is not None and len(si.on_wait) > cap:
                    changed = True
                    waits = list(si.on_wait)
                    for w in waits[:-cap]:
                        out.append(_make_wait_nop(nc, ins.engine, w))
                    ins.sync_info = bass_rust.SyncInfo(
                        on_wait=waits[-cap:], on_update=list(si.on_update)
                    )
                out.append(ins)
            if changed:
                bb.instructions = out


# ---------------------------------------------------------------------------
# Device program
# ---------------------------------------------------------------------------
_NC_CACHE = {}
ANCHORS = np.array([[18.3, 21.6], [60.0, 66.0], [106.8, 175.5],
                    [252.2, 112.9], [312.7, 293.4]], np.float32)


def _build_nc(S, geo):
    nc = bass.Bass()
    slab = nc.dram_tensor("slab", [128, 4, 320], F16, kind="ExternalInput")
    gtt = nc.dram_tensor("gtt", [128, 5 * S], F32, kind="ExternalInput")
    vout = nc.dram_tensor("vout", [128, 320], F16, kind="ExternalOutput")

    with tile.TileContext(nc) as tc:
        with (
            tc.tile_pool(name="cpool", bufs=1) as cpool,
            tc.tile_pool(name="wpool", bufs=1) as wpool,
            tc.tile_pool(name="lpool", bufs=4) as lpool,
        ):
            # ---- ACT table warm-up first: junk memset (gpsimd) + dummy tanh
            # (Tanh/Exp/Relu share one table set -> single early load) ----
            junk = cpool.tile([128, 1], F16)
            junko = cpool.tile([128, 1], F16)
            nc.gpsimd.memset(junk[:], 0.0)
            nc.scalar.activation(junko[:], junk[:], AF.Tanh)

            # ---- input DMAs: planes TX,TY,TW,TH split by partition half ----
            # HWDGE queues are SP + Activation; ACT issues after its dummy.
            T16 = wpool.tile([128, 4 * 320], F16)
            for c in range(4):
                lo = T16[0:64, 320 * c:320 * (c + 1)]
                hi = T16[64:128, 320 * c:320 * (c + 1)]
                nc.sync.dma_start(lo, slab[0:64, c, :])
                nc.scalar.dma_start(hi, slab[64:128, c, :])

            # ---- on-device consts (gpsimd) + gt table DMA (swdge) ----
            XOFFW = cpool.tile([128, 320], F16)
            AW2W = cpool.tile([128, 320], F16)
            AH2W = cpool.tile([128, 320], F16)
            # x-32 per column group (ints, exact in fp16); the 0.5 sigmoid
            # shift is folded into the host gx scalars (as on the y side).
            nc.gpsimd.iota(XOFFW[:], [[1, 64], [0, 5]], base=-32,
                           channel_multiplier=0,
                           allow_small_or_imprecise_dtypes=True)
            GTT = cpool.tile([128, 5 * S], F32)
            nc.gpsimd.dma_start(GTT[:], gtt[:])
            aw = (ANCHORS[:, 0] / 32.0).astype(np.float32)
            ah = (ANCHORS[:, 1] / 32.0).astype(np.float32)
            AW2v = AW2W[:].rearrange("p (x a) -> p a x", a=A)
            AH2v = AH2W[:].rearrange("p (x a) -> p a x", a=A)
            for a in range(A):
                nc.gpsimd.memset(AW2v[:, a, :], float(aw[a]))
                nc.gpsimd.memset(AH2v[:, a, :], float(ah[a]))
            VM = wpool.tile([128, 320], F16)
            nc.vector.memset(VM[:], VM_INIT)

            TX = T16[:, 0:320]
            TY = T16[:, 320:640]
            TW = T16[:, 640:960]
            TH = T16[:, 960:1280]

            # ---- decode: sigmoid via tanh (one ACT table set for all fns) ----
            # sig(t) = 0.5*tanh(0.5 t) + 0.5; the 0.5-shift is folded into
            # XOFFH (x side) and the host gt scalars (y side).
            TXh = wpool.tile([128, 320], F16)
            TYh = wpool.tile([128, 320], F16)
            E0 = wpool.tile([128, 320], F16)
            E1 = wpool.tile([128, 320], F16)
            nc.scalar.activation(TXh[:], TX, AF.Tanh, scale=0.5)
            nc.scalar.activation(TYh[:], TY, AF.Tanh, scale=0.5)
            nc.scalar.activation(E0[:], TW, AF.Exp)
            nc.scalar.activation(E1[:], TH, AF.Exp)

            EW = wpool.tile([128, 320], F16)
            EH = wpool.tile([128, 320], F16)
            SXO = wpool.tile([128, 320], F16)
            NX1 = wpool.tile([128, 320], F16)
            PX2 = wpool.tile([128, 320], F16)
            NY1 = wpool.tile([128, 320], F16)
            PY2 = wpool.tile([128, 320], F16)
            NPA = wpool.tile([128, 320], F16)
            # SXO' = 0.5*TXh + (x-32) = sig(tx) + x - 32 - 0.5 (host gx
            # scalars carry the -0.5 shift, mirroring the y side)
            nc.vector.scalar_tensor_tensor(SXO[:], TXh[:], 0.5, XOFFW[:], OP.mult, OP.add)
            nc.vector.tensor_mul(EW[:], E0[:], AW2W[:])
            nc.vector.tensor_sub(NX1[:], EW[:], SXO[:])
            nc.vector.tensor_add(PX2[:], SXO[:], EW[:])
            nc.vector.tensor_mul(EH[:], E1[:], AH2W[:])
            # y side carries a -0.5 shift (folded into host gy scalars):
            # PY2' = EH + 0.5*TYh = py2 - 0.5 ; NY1' = EH - 0.5*TYh = ny1 + 0.5
            nc.vector.scalar_tensor_tensor(PY2[:], TYh[:], 0.5, EH[:], OP.mult, OP.add)
            nc.vector.scalar_tensor_tensor(NY1[:], TYh[:], -0.5, EH[:], OP.mult, OP.add)
            # npa = -1.5 * EW * EH  ( = -0.375 * pa, pa = 4*EW*EH )
            nc.vector.scalar_tensor_tensor(NPA[:], EW[:], -1.5, EH[:], OP.mult, OP.mult)

            def gcol(k, s):
                return GTT[:, k * S + s: k * S + s + 1]

            def win(t, s):
                xlo, wdt = geo[s]
                return t[:, 5 * xlo: 5 * (xlo + wdt)]

            # ---- slot loop (software pipelined) ----
            st1, st2, st3 = {}, {}, {}

            def emit_stage1(s):
                fd = 5 * geo[s][1]
                r1x = lpool.tile([128, fd], F16, name=f"r1x_{s}", tag="r1x")
                r1y = lpool.tile([128, fd], F16, name=f"r1y_{s}", tag="r1y")
                u = lpool.tile([128, fd], F16, name=f"u_{s}", tag="u")
                v = lpool.tile([128, fd], F16, name=f"v_{s}", tag="v")
                nc.scalar.activation(r1x[:], win(PX2, s), AF.Relu, bias=gcol(0, s), scale=-1.0)
                nc.scalar.activation(r1y[:], win(PY2, s), AF.Relu, bias=gcol(2, s), scale=-1.0)
                nc.vector.tensor_scalar(u[:], win(NX1, s), gcol(1, s), gcol(0, s), OP.min, OP.add)
                nc.vector.tensor_scalar(v[:], win(NY1, s), gcol(3, s), gcol(2, s), OP.min, OP.add)
                st1[s] = (r1x, r1y, u, v)

            def emit_stage2(s):
                r1x, r1y, u, v = st1.pop(s)
                fd = 5 * geo[s][1]
                dx = lpool.tile([128, fd], F16, name=f"dx_{s}", tag="dx")
                dy = lpool.tile([128, fd], F16, name=f"dy_{s}", tag="dy")
                nc.gpsimd.tensor_sub(dx[:], u[:], r1x[:])
                nc.gpsimd.tensor_sub(dy[:], v[:], r1y[:])
                st2[s] = (dx, dy)

            def emit_stage3(s):
                dx, dy = st2.pop(s)
                fd = 5 * geo[s][1]
                iv = lpool.tile([128, fd], F16, name=f"iv_{s}", tag="iv")
                nc.vector.scalar_tensor_tensor(iv[:], dx[:], 0.0, dy[:], OP.max, OP.mult)
                st3[s] = iv

            def emit_stage4(s):
                iv = st3.pop(s)
                nc.vector.scalar_tensor_tensor(
                    win(VM, s), iv[:], gcol(4, s), win(VM, s), OP.add, OP.max
                )

            for s in range(S + 3):
                if s < S:
                    emit_stage1(s)
                if s >= 1 and s - 1 < S:
                    emit_stage2(s - 1)
                if s >= 2 and s - 2 < S:
                    emit_stage3(s - 2)
                if s >= 3:
                    emit_stage4(s - 3)

            VR = wpool.tile([128, 320], F16)
            nc.vector.tensor_add(VR[:], VM[:], NPA[:])
            nc.sync.dma_start(vout[0:43, :], VR[0:43, :])
            nc.scalar.dma_start(vout[43:86, :], VR[43:86, :])
            nc.gpsimd.dma_start(vout[86:128, :], VR[86:128, :])

    _split_multiwait(nc)
    return nc


def _get_nc(S, geo):
    key = (S, tuple(geo))
    if key not in _NC_CACHE:
        _NC_CACHE[key] = _build_nc(S, geo)
    return _NC_CACHE[key]


# ---------------------------------------------------------------------------
# Host: geometry, packing, tables
# ---------------------------------------------------------------------------
def _gt_geom(target):
    tgt = target.astype(np.float32)
    inv16 = np.float32(1.0 / 16.0)
    cx = tgt[:, :, 1] * inv16
    cy = tgt[:, :, 2] * inv16
    w = tgt[:, :, 3] * inv16
    h = tgt[:, :, 4] * inv16
    gx1 = cx - w * np.float32(0.5)
    gx2 = cx + w * np.float32(0.5)
    gy1 = cy - h * np.float32(0.5)
    gy2 = cy + h * np.float32(0.5)
    ga = w * h
    return gx1, gx2, gy1, gy2, w, h, ga


def _delta(w, h, ga):
    # fp16 pipeline rounding (~0.05*(gw+gh)) + ACT table slop + pa-term error.
    return (np.float32(0.05) * (w + h) + np.float32(0.016) * ga
            + np.float32(0.02)).astype(np.float32)


def _pack(items, wgrow_cap):
    items = sorted(items, key=lambda t: (t[5], t[6]))
    slots = []
    for it in items:
        core, b, n, y0, y1, x0, x1 = it
        mask = ((1 << (y1 - y0 + 1)) - 1) << (64 * b + y0)
        best, best_cost = -1, None
        for si, sl in enumerate(slots):
            if sl[2].get(core, 0) & mask:
                continue
            grow = max(sl[1], x1) - min(sl[0], x0) - (sl[1] - sl[0])
            if best_cost is None or grow < best_cost:
                best, best_cost = si, grow
        if best < 0 or best_cost > wgrow_cap:
            slots.append([x0, x1, {core: mask}, [it]])
        else:
            sl = slots[best]
            sl[0] = min(sl[0], x0)
            sl[1] = max(sl[1], x1)
            sl[2][core] = sl[2].get(core, 0) | mask
            sl[3].append(it)
    return [(sl[0], sl[1], sl[3]) for sl in slots]


def _host_tables(target, anchors):
    gx1, gx2, gy1, gy2, w, h, ga = _gt_geom(target)
    delta = _delta(w, h, ga)
    d_rel = delta / ga
    E = np.maximum(
        0.0,
        np.float32(0.125) * np.minimum(np.float32(2.667),
                                       np.float32(1.667) + np.float32(2.667) * d_rel)
        - np.float32(T375) + d_rel,
    ).astype(np.float32)
    PAD = 0.10

    def cell_range(lo, hi, ext):
        c0 = np.clip(np.floor(lo - ext - PAD + 1.0) - 1.0, 0, 63).astype(np.int64)
        c1 = np.clip(np.ceil(hi + ext + PAD) - 1.0, 0, 63).astype(np.int64)
        return c0, np.maximum(c1, c0)

    y0c, y1c = cell_range(gy1, gy2, E * h)
    x0c, x1c = cell_range(gx1, gx2, E * w)

    items = []
    for i in range(NCORES):
        for b in range(BPC):
            g = 2 * i + b
            for n in range(N):
                items.append((i, b, n, int(y0c[g, n]), int(y1c[g, n]),
                              int(x0c[g, n]), int(x1c[g, n])))
    best = None
    for cap in (24, 32, 48):
        slots = _pack(items, cap)
        cost = sum(min(64, ((sl[1] - sl[0] + 1 + 15) // 16) * 16) + 32 for sl in slots)
        if best is None or cost < best[0]:
            best = (cost, slots)
    slots = best[1]
    S = len(slots)
    geo = []
    for (xlo, xhi, _) in slots:
        xlo2 = (xlo // 8) * 8
        wdt = min(64 - xlo2, ((xhi - xlo2 + 1 + 15) // 16) * 16)
        geo.append((int(xlo2), int(wdt)))

    c5 = (-np.float32(T375) * ga + delta).astype(np.float32)
    yrow = np.arange(64, dtype=np.float32)
    gtts = [np.zeros((128, 5 * S), np.float32) for _ in range(NCORES)]
    for i in range(NCORES):
        gtts[i][:, 4 * S:5 * S] = NULL_C5
    for s, (_, _, members) in enumerate(slots):
        for (i, b, n, r0, r1, _, _) in members:
            g = 2 * i + b
            rows = slice(64 * b + r0, 64 * b + r1 + 1)
            yv = yrow[r0:r1 + 1]
            # both axes: device PX2/PY2 carry a -0.5 shift (tanh-sigmoid fold)
            gtts[i][rows, 0 * S + s] = gx2[g, n] - XSHIFT - 0.5
            gtts[i][rows, 1 * S + s] = -(gx1[g, n] - XSHIFT) + 0.5
            gtts[i][rows, 2 * S + s] = gy2[g, n] - yv - 0.5
            gtts[i][rows, 3 * S + s] = yv - gy1[g, n] + 0.5
            gtts[i][rows, 4 * S + s] = c5[g, n]
    return gtts, S, geo


def _prep_slabs(output):
    """(16,425,64,64) f32 -> per-core [128, 4, 320] fp16, planes TX,TY,TW,TH,
    free dim (x,a)."""
    o = output.reshape(B, A, 85, H, W)[:, :, :4]        # (B, a, c, y, x)
    arr = np.ascontiguousarray(o.transpose(0, 3, 2, 4, 1))  # (B, y, c, x, a)
    arr = arr.reshape(B, 64, 4, 320).astype(np.float16)
    return [np.ascontiguousarray(
        np.concatenate([arr[2 * i], arr[2 * i + 1]], axis=0)) for i in range(NCORES)]


# ---------------------------------------------------------------------------
# Exact host tail (same as v1)
# ---------------------------------------------------------------------------
def _sigmoid32(x):
    return np.float32(1.0) / (np.float32(1.0) + np.exp(-x, dtype=np.float32))


def _exact_candidates(output, target, anchors, cand_idx):
    bg, aa, yy, xx = cand_idx
    if bg.shape[0] == 0:
        z = np.zeros(0)
        return z.astype(bool), z.astype(np.int64)

    out = output
    tx = out[bg, 85 * aa + 0, yy, xx]
    ty = out[bg, 85 * aa + 1, yy, xx]
    tw = out[bg, 85 * aa + 2, yy, xx]
    th = out[bg, 85 * aa + 3, yy, xx]
    an = anchors.astype(np.float32)
    px = (_sigmoid32(tx) + xx.astype(np.float32)) * np.float32(STRIDE)
    py = (_sigmoid32(ty) + yy.astype(np.float32)) * np.float32(STRIDE)
    pw = np.exp(tw, dtype=np.float32) * an[aa, 0]
    ph = np.exp(th, dtype=np.float32) * an[aa, 1]

    g = target[:, :, 1:].astype(np.float32)
    gx1 = g[:, :, 0] - g[:, :, 2] * np.float32(0.5)
    gx2 = g[:, :, 0] + g[:, :, 2] * np.float32(0.5)
    gy1 = g[:, :, 1] - g[:, :, 3] * np.float32(0.5)
    gy2 = g[:, :, 1] + g[:, :, 3] * np.float32(0.5)
    g_area = (gx2 - gx1) * (gy2 - gy1)

    px1 = px - pw * np.float32(0.5)
    px2 = px + pw * np.float32(0.5)
    py1 = py - ph * np.float32(0.5)
    py2 = py + ph * np.float32(0.5)
    p_area = (px2 - px1) * (py2 - py1)

    ix1 = np.maximum(gx1[bg], px1[:, None])
    iy1 = np.maximum(gy1[bg], py1[:, None])
    ix2 = np.minimum(gx2[bg], px2[:, None])
    iy2 = np.minimum(gy2[bg], py2[:, None])
    inter = np.clip(ix2 - ix1, 0, None) * np.clip(iy2 - iy1, 0, None)
    union = g_area[bg] + p_area[:, None] - inter + np.float32(1e-6)
    iou = inter / union
    best = iou.max(axis=1)
    bidx = iou.argmax(axis=1)
    return best > np.float32(THRESH), bidx


def _build_run_args(output, target, anchors):
    """(nc, in_maps) for the device run — shared with the test harness."""
    gtts, S, geo = _host_tables(target, anchors)
    slabs = _prep_slabs(output)
    nc = _get_nc(S, geo)
    in_maps = [{"slab": slabs[i], "gtt": gtts[i]} for i in range(NCORES)]
    return nc, in_maps


LAST_VMAX = None  # test-harness introspection


def kernel(output, target, anchors):
    global LAST_VMAX
    output = np.ascontiguousarray(output, np.float32)
    target = np.ascontiguousarray(target, np.float32)
    anchors = np.ascontiguousarray(anchors, np.float32)

    nc, in_maps = _build_run_args(output, target, anchors)
    res = run_bass_kernel_spmd(nc, in_maps, list(range(NCORES)))

    vmax = np.zeros((B, A, H, W), np.float32)
    for i in range(NCORES):
        vo = res.results[i]["vout"].astype(np.float32)
        for b in range(BPC):
            g = 2 * i + b
            vmax[g] = (
                vo[64 * b:64 * b + 64, :].reshape(64, 64, 5).transpose(2, 0, 1)
            )
    LAST_VMAX = vmax

    cand = vmax > 0.0
    bg, aa, yy, xx = np.nonzero(cand)
    mask_c, bidx_c = _exact_candidates(output, target, anchors, (bg, aa, yy, xx))

    m = mask_c
    bgm, aam, yym, xxm = bg[m], aa[m], yy[m], xx[m]
    idxm = bidx_c[m]

    coord_loss = 0.0
    if bgm.size:
        d = 0.0
        for c in range(4):
            pc = output[bgm, 85 * aam + c, yym, xxm].astype(np.float64)
            tc = target[bgm, idxm, 1 + c].astype(np.float64)
            d += np.sum((pc - tc) ** 2)
        coord_loss = d

    conf_all = output[:, 4::85, :, :].astype(np.float64)
    conf_loss = np.sum(conf_all * conf_all)
    if bgm.size:
        cm = output[bgm, 85 * aam + 4, yym, xxm].astype(np.float64)
        conf_loss += np.sum(25.0 * (cm - 1.0) ** 2 - cm * cm)

    cls_loss = 0.0
    if bgm.size:
        ch = (85 * aam[:, None] + 5 + np.arange(C)[None, :])
        logits = output[bgm[:, None], ch, yym[:, None], xxm[:, None]].astype(np.float64)
        lse = np.log(np.sum(np.exp(logits), axis=1))
        tcls = target[bgm, idxm, 0].astype(np.int64)
        logit_sel = logits[np.arange(bgm.size), tcls]
        cls_loss = np.sum(lse - logit_sel)

    total = coord_loss + conf_loss + cls_loss
    return np.float32(total)


# revision 11
# speedup vs baseline: 1.1159x; 1.0380x over previous
"""RegionLoss (YOLOv2) filter kernel v2 — fp16 datapath, (x,a) layout.

Shapes: output (16,425,64,64) f32, target (16,50,5) f32, anchors (5,2) f32.
A=5, C=80, H=W=64, N=50, STRIDE=16. 8 cores, 2 batches each.

Device computes a conservative candidate filter vres[p=(b,y), f=(x,a)]:
  vres = max_n [ relu(dx_n)*dy_n + c5_n ] - 0.375*pa  (>0 => candidate)
with per-gt row-packed slots (S slots, x-windows). Host does the exact
fp32 tail (iou/argmax/loss) on the ~2k candidates.

v2 vs v1: host pre-packs the 20 coord channels to fp16 in device layout
(halves DMA bytes, makes everything contiguous); free dim is (x,a) so slot
windows are contiguous (fp16 2x/4x DVE modes engage); per-slot work is
spread ACT/DVE/GPSIMD; consts built on-device (iota/memset, no 491KB DMA);
DMAs issued from 3 queues; output DMA split 4 ways.
"""

import os
import numpy as np

import concourse.bass as bass
import concourse.mybir as mybir
from concourse import tile
from concourse.bass_utils import run_bass_kernel_spmd
from concourse.vector_clock import ScopedClock
import bass_rust

F32 = mybir.dt.float32
F16 = mybir.dt.float16
OP = mybir.AluOpType
AF = mybir.ActivationFunctionType

A, C, H, W, N = 5, 80, 64, 64, 50
B = 16
NCORES = 8
BPC = B // NCORES
STRIDE = 16.0
THRESH = 0.6
T375 = THRESH / (1.0 + THRESH)
NULL_C5 = -1.0e9
XSHIFT = 32.0
VM_INIT = -60000.0


# ---------------------------------------------------------------------------
# Tile tail-drain patch + multi-wait splitting (same as v1): cheap teardown.
# ---------------------------------------------------------------------------
def _patched_drain_and_barrier(self, tick_clock, wait_clock):
    nc = self.nc
    drain_inst = nc.sync.drain()
    wait_clock.add_sem_waits(drain_inst.ins, ScopedClock({None: tick_clock.global_clock}))
    si = drain_inst.ins.sync_info
    if si is not None and len(si.on_wait) > 1:
        waits = list(si.on_wait)
        drain_inst.ins.sync_info = bass_rust.SyncInfo(
            on_wait=[waits[0]], on_update=list(si.on_update)
        )
        for w in waits[1:]:
            nop = nc.sync.nop(nofuse=True)
            nop.ins.sync_info = bass_rust.SyncInfo(on_wait=[w], on_update=[])

    assert self.sems is not None
    popped = nc._tile_sem_poison_stack.pop()
    assert popped is self._sem_poison

    from concourse.bass import compact_to_ranges

    sems = list(self.sems.allocated().values())
    if sems:
        hs = nc._state.alloc_semaphore(name="td_hs")
        nc.sync.sem_inc(hs, 1)
        nc.gpsimd.wait_ge(hs, 1)
        sem_nums = [s.num if hasattr(s, "num") else s for s in sems] + [
            hs.num if hasattr(hs, "num") else hs
        ]
        for sem_range in compact_to_ranges(sorted(sem_nums)):
            nc.gpsimd.dma_reset(sem_range)
            nc.gpsimd.sem_clear(sem_range)
        nc._state.prepend_free_semaphores(sem_nums)
        for poison_set in nc._tile_sem_poison_stack:
            poison_set.update(sem_nums)


if not os.environ.get("K2_NO_PATCH") and getattr(tile.TileContext, "_drain_patch", None) is None:
    tile.TileContext._drain_and_barrier = _patched_drain_and_barrier
    tile.TileContext._drain_patch = True


def _make_wait_nop(nc, engine_type, w):
    nop = nc.engines[engine_type].nop(nofuse=True)
    inst = nop.ins
    cur = nc.cur_bb.bb
    lst = list(cur.instructions)
    assert lst and lst[-1].name == inst.name, "nop not at tail of cur_bb"
    cur.instructions = lst[:-1]
    inst.sync_info = bass_rust.SyncInfo(on_wait=[w], on_update=[])
    return inst


def _split_multiwait(nc):
    for f in nc.m.functions:
        for bb in f.blocks:
            insts = list(bb.instructions)
            out = []
            changed = False
            for ins in insts:
                si = ins.sync_info
                cap = 2 if isinstance(ins, mybir.InstEventSemaphore) else 1
                if si is not None and len(si.on_wait) > cap:
                    changed = True
                    waits = list(si.on_wait)
                    for w in waits[:-cap]:
                        out.append(_make_wait_nop(nc, ins.engine, w))
                    ins.sync_info = bass_rust.SyncInfo(
                        on_wait=waits[-cap:], on_update=list(si.on_update)
                    )
                out.append(ins)
            if changed:
                bb.instructions = out


# ---------------------------------------------------------------------------
# Device program
# ---------------------------------------------------------------------------
_NC_CACHE = {}
ANCHORS = np.array([[18.3, 21.6], [60.0, 66.0], [106.8, 175.5],
                    [252.2, 112.9], [312.7, 293.4]], np.float32)


def _build_nc(S, geo):
    nc = bass.Bass()
    slab = nc.dram_tensor("slab", [128, 4, 320], F16, kind="ExternalInput")
    gtt = nc.dram_tensor("gtt", [128, 5 * S], F32, kind="ExternalInput")
    vout = nc.dram_tensor("vout", [128, 320], F16, kind="ExternalOutput")

    with tile.TileContext(nc) as tc:
        with (
            tc.tile_pool(name="cpool", bufs=1) as cpool,
            tc.tile_pool(name="wpool", bufs=1) as wpool,
            tc.tile_pool(name="lpool", bufs=8) as lpool,
        ):
            # ---- ACT table warm-up first: junk memset (gpsimd) + dummy tanh
            # (Tanh/Exp/Relu share one table set -> single early load) ----
            junk = cpool.tile([128, 1], F16)
            junko = cpool.tile([128, 1], F16)
            nc.gpsimd.memset(junk[:], 0.0)
            nc.scalar.activation(junko[:], junk[:], AF.Tanh)

            # ---- input DMAs: planes TX,TY,TW,TH split by partition half ----
            # HWDGE queues are SP + Activation; ACT issues after its dummy.
            # gt tables first on SP (needed by the first slot ops).
            GTT = cpool.tile([128, 5 * S], F32)
            nc.sync.dma_start(GTT[:], gtt[:])
            T16 = wpool.tile([128, 4 * 320], F16)
            for c in range(4):
                lo = T16[0:64, 320 * c:320 * (c + 1)]
                hi = T16[64:128, 320 * c:320 * (c + 1)]
                nc.sync.dma_start(lo, slab[0:64, c, :])
                nc.scalar.dma_start(hi, slab[64:128, c, :])

            # ---- on-device consts (gpsimd) ----
            XOFFW = cpool.tile([128, 320], F16)
            AW2W = cpool.tile([128, 320], F16)
            AH2W = cpool.tile([128, 320], F16)
            # x-32 per column group (ints, exact in fp16); the 0.5 sigmoid
            # shift is folded into the host gx scalars (as on the y side).
            nc.gpsimd.iota(XOFFW[:], [[1, 64], [0, 5]], base=-32,
                           channel_multiplier=0,
                           allow_small_or_imprecise_dtypes=True)
            aw = (ANCHORS[:, 0] / 32.0).astype(np.float32)
            ah = (ANCHORS[:, 1] / 32.0).astype(np.float32)
            AW2v = AW2W[:].rearrange("p (x a) -> p a x", a=A)
            AH2v = AH2W[:].rearrange("p (x a) -> p a x", a=A)
            for a in range(A):
                nc.gpsimd.memset(AW2v[:, a, :], float(aw[a]))
                nc.gpsimd.memset(AH2v[:, a, :], float(ah[a]))
            VM = wpool.tile([128, 320], F16)
            nc.vector.memset(VM[:], VM_INIT)

            TX = T16[:, 0:320]
            TY = T16[:, 320:640]
            TW = T16[:, 640:960]
            TH = T16[:, 960:1280]

            # ---- decode: sigmoid via tanh (one ACT table set for all fns) ----
            # sig(t) = 0.5*tanh(0.5 t) + 0.5; the 0.5-shift is folded into
            # XOFFH (x side) and the host gt scalars (y side).
            TXh = wpool.tile([128, 320], F16)
            TYh = wpool.tile([128, 320], F16)
            E0 = wpool.tile([128, 320], F16)
            E1 = wpool.tile([128, 320], F16)
            nc.scalar.activation(TXh[:], TX, AF.Tanh, scale=0.5)
            nc.scalar.activation(TYh[:], TY, AF.Tanh, scale=0.5)
            nc.scalar.activation(E0[:], TW, AF.Exp)
            nc.scalar.activation(E1[:], TH, AF.Exp)

            EW = wpool.tile([128, 320], F16)
            EH = wpool.tile([128, 320], F16)
            SXO = wpool.tile([128, 320], F16)
            NX1 = wpool.tile([128, 320], F16)
            PX2 = wpool.tile([128, 320], F16)
            NY1 = wpool.tile([128, 320], F16)
            PY2 = wpool.tile([128, 320], F16)
            NPA = wpool.tile([128, 320], F16)
            # SXO' = 0.5*TXh + (x-32) = sig(tx) + x - 32 - 0.5 (host gx
            # scalars carry the -0.5 shift, mirroring the y side)
            nc.vector.scalar_tensor_tensor(SXO[:], TXh[:], 0.5, XOFFW[:], OP.mult, OP.add)
            nc.vector.tensor_mul(EW[:], E0[:], AW2W[:])
            nc.vector.tensor_sub(NX1[:], EW[:], SXO[:])
            nc.vector.tensor_add(PX2[:], SXO[:], EW[:])
            nc.vector.tensor_mul(EH[:], E1[:], AH2W[:])
            # y side carries a -0.5 shift (folded into host gy scalars):
            # PY2' = EH + 0.5*TYh = py2 - 0.5 ; NY1' = EH - 0.5*TYh = ny1 + 0.5
            nc.vector.scalar_tensor_tensor(PY2[:], TYh[:], 0.5, EH[:], OP.mult, OP.add)
            nc.vector.scalar_tensor_tensor(NY1[:], TYh[:], -0.5, EH[:], OP.mult, OP.add)
            # npa = -1.5 * EW * EH  ( = -0.375 * pa, pa = 4*EW*EH )
            nc.vector.scalar_tensor_tensor(NPA[:], EW[:], -1.5, EH[:], OP.mult, OP.mult)

            def gcol(k, s):
                return GTT[:, k * S + s: k * S + s + 1]

            def win(t, s):
                xlo, wdt = geo[s]
                return t[:, 5 * xlo: 5 * (xlo + wdt)]

            # ---- slot loop (software pipelined) ----
            # stage1: ACT r1x/r1y, DVE u/v.  stage2: GPS dx/dy (in-place over
            # u/v).  stage3: ACT rdx=relu(dx), stage4: DVE iv = rdx*dy,
            # stage5: DVE vm max-accumulate.
            st1, st2, st3, st4 = {}, {}, {}, {}

            def emit_stage1(s):
                fd = 5 * geo[s][1]
                r1x = lpool.tile([128, fd], F16, name=f"r1x_{s}", tag="r1x")
                r1y = lpool.tile([128, fd], F16, name=f"r1y_{s}", tag="r1y")
                u = lpool.tile([128, fd], F16, name=f"u_{s}", tag="u")
                v = lpool.tile([128, fd], F16, name=f"v_{s}", tag="v")
                nc.scalar.activation(r1x[:], win(PX2, s), AF.Relu, bias=gcol(0, s), scale=-1.0)
                nc.scalar.activation(r1y[:], win(PY2, s), AF.Relu, bias=gcol(2, s), scale=-1.0)
                nc.vector.tensor_scalar(u[:], win(NX1, s), gcol(1, s), gcol(0, s), OP.min, OP.add)
                nc.vector.tensor_scalar(v[:], win(NY1, s), gcol(3, s), gcol(2, s), OP.min, OP.add)
                st1[s] = (r1x, r1y, u, v)

            def emit_stage2(s):
                r1x, r1y, u, v = st1.pop(s)
                nc.gpsimd.tensor_sub(u[:], u[:], r1x[:])   # u <- dx
                nc.gpsimd.tensor_sub(v[:], v[:], r1y[:])   # v <- dy
                st2[s] = (u, v)

            def emit_stage3(s):
                dx, dy = st2.pop(s)
                fd = 5 * geo[s][1]
                rdx = lpool.tile([128, fd], F16, name=f"rdx_{s}", tag="rdx")
                nc.scalar.activation(rdx[:], dx[:], AF.Relu)
                st3[s] = (rdx, dy)

            def emit_stage4(s):
                rdx, dy = st3.pop(s)
                fd = 5 * geo[s][1]
                iv = lpool.tile([128, fd], F16, name=f"iv_{s}", tag="iv")
                nc.vector.tensor_mul(iv[:], rdx[:], dy[:])
                st4[s] = iv

            def emit_stage5(s):
                iv = st4.pop(s)
                nc.vector.scalar_tensor_tensor(
                    win(VM, s), iv[:], gcol(4, s), win(VM, s), OP.add, OP.max
                )

            for s in range(S + 5):
                if s < S:
                    emit_stage1(s)
                if 0 <= s - 1 < S:
                    emit_stage2(s - 1)
                if 0 <= s - 2 < S:
                    emit_stage3(s - 2)
                if 0 <= s - 3 < S:
                    emit_stage4(s - 3)
                if 0 <= s - 4 < S:
                    emit_stage5(s - 4)

            VR = wpool.tile([128, 320], F16)
            nc.vector.tensor_add(VR[:], VM[:], NPA[:])
            nc.sync.dma_start(vout[0:43, :], VR[0:43, :])
            nc.scalar.dma_start(vout[43:86, :], VR[43:86, :])
            nc.gpsimd.dma_start(vout[86:128, :], VR[86:128, :])

    _split_multiwait(nc)
    return nc


def _get_nc(S, geo):
    key = (S, tuple(geo))
    if key not in _NC_CACHE:
        _NC_CACHE[key] = _build_nc(S, geo)
    return _NC_CACHE[key]


# ---------------------------------------------------------------------------
# Host: geometry, packing, tables
# ---------------------------------------------------------------------------
def _gt_geom(target):
    tgt = target.astype(np.float32)
    inv16 = np.float32(1.0 / 16.0)
    cx = tgt[:, :, 1] * inv16
    cy = tgt[:, :, 2] * inv16
    w = tgt[:, :, 3] * inv16
    h = tgt[:, :, 4] * inv16
    gx1 = cx - w * np.float32(0.5)
    gx2 = cx + w * np.float32(0.5)
    gy1 = cy - h * np.float32(0.5)
    gy2 = cy + h * np.float32(0.5)
    ga = w * h
    return gx1, gx2, gy1, gy2, w, h, ga


def _delta(w, h, ga):
    # fp16 pipeline rounding (~0.05*(gw+gh)) + ACT table slop + pa-term error.
    return (np.float32(0.05) * (w + h) + np.float32(0.016) * ga
            + np.float32(0.02)).astype(np.float32)


def _pack(items, wgrow_cap):
    items = sorted(items, key=lambda t: (t[5], t[6]))
    slots = []
    for it in items:
        core, b, n, y0, y1, x0, x1 = it
        mask = ((1 << (y1 - y0 + 1)) - 1) << (64 * b + y0)
        best, best_cost = -1, None
        for si, sl in enumerate(slots):
            if sl[2].get(core, 0) & mask:
                continue
            grow = max(sl[1], x1) - min(sl[0], x0) - (sl[1] - sl[0])
            if best_cost is None or grow < best_cost:
                best, best_cost = si, grow
        if best < 0 or best_cost > wgrow_cap:
            slots.append([x0, x1, {core: mask}, [it]])
        else:
            sl = slots[best]
            sl[0] = min(sl[0], x0)
            sl[1] = max(sl[1], x1)
            sl[2][core] = sl[2].get(core, 0) | mask
            sl[3].append(it)
    return [(sl[0], sl[1], sl[3]) for sl in slots]


def _host_tables(target, anchors):
    gx1, gx2, gy1, gy2, w, h, ga = _gt_geom(target)
    delta = _delta(w, h, ga)
    d_rel = delta / ga
    E = np.maximum(
        0.0,
        np.float32(0.125) * np.minimum(np.float32(2.667),
                                       np.float32(1.667) + np.float32(2.667) * d_rel)
        - np.float32(T375) + d_rel,
    ).astype(np.float32)
    PAD = 0.10

    def cell_range(lo, hi, ext):
        c0 = np.clip(np.floor(lo - ext - PAD + 1.0) - 1.0, 0, 63).astype(np.int64)
        c1 = np.clip(np.ceil(hi + ext + PAD) - 1.0, 0, 63).astype(np.int64)
        return c0, np.maximum(c1, c0)

    y0c, y1c = cell_range(gy1, gy2, E * h)
    x0c, x1c = cell_range(gx1, gx2, E * w)

    items = []
    for i in range(NCORES):
        for b in range(BPC):
            g = 2 * i + b
            for n in range(N):
                items.append((i, b, n, int(y0c[g, n]), int(y1c[g, n]),
                              int(x0c[g, n]), int(x1c[g, n])))
    best = None
    for cap in (24, 32, 48):
        slots = _pack(items, cap)
        cost = sum(min(64, ((sl[1] - sl[0] + 1 + 15) // 16) * 16) + 32 for sl in slots)
        if best is None or cost < best[0]:
            best = (cost, slots)
    slots = best[1]
    S = len(slots)
    geo = []
    for (xlo, xhi, _) in slots:
        xlo2 = (xlo // 8) * 8
        wdt = min(64 - xlo2, ((xhi - xlo2 + 1 + 15) // 16) * 16)
        geo.append((int(xlo2), int(wdt)))

    c5 = (-np.float32(T375) * ga + delta).astype(np.float32)
    yrow = np.arange(64, dtype=np.float32)
    gtts = [np.zeros((128, 5 * S), np.float32) for _ in range(NCORES)]
    for i in range(NCORES):
        gtts[i][:, 4 * S:5 * S] = NULL_C5
    for s, (_, _, members) in enumerate(slots):
        for (i, b, n, r0, r1, _, _) in members:
            g = 2 * i + b
            rows = slice(64 * b + r0, 64 * b + r1 + 1)
            yv = yrow[r0:r1 + 1]
            # both axes: device PX2/PY2 carry a -0.5 shift (tanh-sigmoid fold)
            gtts[i][rows, 0 * S + s] = gx2[g, n] - XSHIFT - 0.5
            gtts[i][rows, 1 * S + s] = -(gx1[g, n] - XSHIFT) + 0.5
            gtts[i][rows, 2 * S + s] = gy2[g, n] - yv - 0.5
            gtts[i][rows, 3 * S + s] = yv - gy1[g, n] + 0.5
            gtts[i][rows, 4 * S + s] = c5[g, n]
    return gtts, S, geo


def _prep_slabs(output):
    """(16,425,64,64) f32 -> per-core [128, 4, 320] fp16, planes TX,TY,TW,TH,
    free dim (x,a)."""
    o = output.reshape(B, A, 85, H, W)[:, :, :4]        # (B, a, c, y, x)
    arr = np.ascontiguousarray(o.transpose(0, 3, 2, 4, 1))  # (B, y, c, x, a)
    arr = arr.reshape(B, 64, 4, 320).astype(np.float16)
    return [np.ascontiguousarray(
        np.concatenate([arr[2 * i], arr[2 * i + 1]], axis=0)) for i in range(NCORES)]


# ---------------------------------------------------------------------------
# Exact host tail (same as v1)
# ---------------------------------------------------------------------------
def _sigmoid32(x):
    return np.float32(1.0) / (np.float32(1.0) + np.exp(-x, dtype=np.float32))


def _exact_candidates(output, target, anchors, cand_idx):
    bg, aa, yy, xx = cand_idx
    if bg.shape[0] == 0:
        z = np.zeros(0)
        return z.astype(bool), z.astype(np.int64)

    out = output
    tx = out[bg, 85 * aa + 0, yy, xx]
    ty = out[bg, 85 * aa + 1, yy, xx]
    tw = out[bg, 85 * aa + 2, yy, xx]
    th = out[bg, 85 * aa + 3, yy, xx]
    an = anchors.astype(np.float32)
    px = (_sigmoid32(tx) + xx.astype(np.float32)) * np.float32(STRIDE)
    py = (_sigmoid32(ty) + yy.astype(np.float32)) * np.float32(STRIDE)
    pw = np.exp(tw, dtype=np.float32) * an[aa, 0]
    ph = np.exp(th, dtype=np.float32) * an[aa, 1]

    g = target[:, :, 1:].astype(np.float32)
    gx1 = g[:, :, 0] - g[:, :, 2] * np.float32(0.5)
    gx2 = g[:, :, 0] + g[:, :, 2] * np.float32(0.5)
    gy1 = g[:, :, 1] - g[:, :, 3] * np.float32(0.5)
    gy2 = g[:, :, 1] + g[:, :, 3] * np.float32(0.5)
    g_area = (gx2 - gx1) * (gy2 - gy1)

    px1 = px - pw * np.float32(0.5)
    px2 = px + pw * np.float32(0.5)
    py1 = py - ph * np.float32(0.5)
    py2 = py + ph * np.float32(0.5)
    p_area = (px2 - px1) * (py2 - py1)

    ix1 = np.maximum(gx1[bg], px1[:, None])
    iy1 = np.maximum(gy1[bg], py1[:, None])
    ix2 = np.minimum(gx2[bg], px2[:, None])
    iy2 = np.minimum(gy2[bg], py2[:, None])
    inter = np.clip(ix2 - ix1, 0, None) * np.clip(iy2 - iy1, 0, None)
    union = g_area[bg] + p_area[:, None] - inter + np.float32(1e-6)
    iou = inter / union
    best = iou.max(axis=1)
    bidx = iou.argmax(axis=1)
    return best > np.float32(THRESH), bidx


def _build_run_args(output, target, anchors):
    """(nc, in_maps) for the device run — shared with the test harness."""
    gtts, S, geo = _host_tables(target, anchors)
    slabs = _prep_slabs(output)
    nc = _get_nc(S, geo)
    in_maps = [{"slab": slabs[i], "gtt": gtts[i]} for i in range(NCORES)]
    return nc, in_maps


LAST_VMAX = None  # test-harness introspection


def kernel(output, target, anchors):
    global LAST_VMAX
    output = np.ascontiguousarray(output, np.float32)
    target = np.ascontiguousarray(target, np.float32)
    anchors = np.ascontiguousarray(anchors, np.float32)

    nc, in_maps = _build_run_args(output, target, anchors)
    res = run_bass_kernel_spmd(nc, in_maps, list(range(NCORES)))

    vmax = np.zeros((B, A, H, W), np.float32)
    for i in range(NCORES):
        vo = res.results[i]["vout"].astype(np.float32)
        for b in range(BPC):
            g = 2 * i + b
            vmax[g] = (
                vo[64 * b:64 * b + 64, :].reshape(64, 64, 5).transpose(2, 0, 1)
            )
    LAST_VMAX = vmax

    cand = vmax > 0.0
    bg, aa, yy, xx = np.nonzero(cand)
    mask_c, bidx_c = _exact_candidates(output, target, anchors, (bg, aa, yy, xx))

    m = mask_c
    bgm, aam, yym, xxm = bg[m], aa[m], yy[m], xx[m]
    idxm = bidx_c[m]

    coord_loss = 0.0
    if bgm.size:
        d = 0.0
        for c in range(4):
            pc = output[bgm, 85 * aam + c, yym, xxm].astype(np.float64)
            tc = target[bgm, idxm, 1 + c].astype(np.float64)
            d += np.sum((pc - tc) ** 2)
        coord_loss = d

    conf_all = output[:, 4::85, :, :].astype(np.float64)
    conf_loss = np.sum(conf_all * conf_all)
    if bgm.size:
        cm = output[bgm, 85 * aam + 4, yym, xxm].astype(np.float64)
        conf_loss += np.sum(25.0 * (cm - 1.0) ** 2 - cm * cm)

    cls_loss = 0.0
    if bgm.size:
        ch = (85 * aam[:, None] + 5 + np.arange(C)[None, :])
        logits = output[bgm[:, None], ch, yym[:, None], xxm[:, None]].astype(np.float64)
        lse = np.log(np.sum(np.exp(logits), axis=1))
        tcls = target[bgm, idxm, 0].astype(np.int64)
        logit_sel = logits[np.arange(bgm.size), tcls]
        cls_loss = np.sum(lse - logit_sel)

    total = coord_loss + conf_loss + cls_loss
    return np.float32(total)
